# revision 31
# baseline (speedup 1.0000x reference)
"""Distributed Bass kernel for nn_AttentionCircuit (B=2,S=2048,D=2048,RANK=512,H=16).

Sharding: 8 cores = 2 batches x 4 group-positions. Core (b, g) computes
attention for head-group g (4 heads / 512 D-cols) of batch b over all S,
then a partial W_O product over its 512 AO columns; a ReduceScatter(add)
per 512-row chunk sums the partials and scatters rows across the group.
All matmul operands bf16 (1 cyc/row on PE, half the DMA bytes of fp32).

Per-core dataflow, streamed per 512-wide s-chunk c (no AllGather: the gated
low-rank t is recomputed locally from x, which keeps the PE continuously
busy instead of idling ~100us on a collective):
  A(c): t^T = read @ x_c^T, gate with g_c^T -> tqg/tkg/tvg [rank, 512]
  B(c): Q^T/K^T[own cols, chunk c], V[chunk c, own cols]
  C(t=c): per head: scores^T = K^T.T Q^T -> exp -> causal mask (block skip +
     static masks on diagonal) -> rowsum via ones-matmul of DVE pair-sums ->
     PV matmul -> normalize with outer(1/0.81, 1/rowsum) bcast matmul
  D(c): partial out rows = ao^T.T @ W_O[own rows, :]
  RS(c): ReduceScatter(add) over the 4-core group -> 128 output rows.
Host reassembles: core (b, g) holds rows t*512+g*128..+128 of batch b.
"""
import sys
import numpy as np
import ml_dtypes

sys.path.insert(0, '/opt/trn_rl_repo')

import concourse.bass as bass  # noqa: E402
from concourse import bacc  # noqa: E402
import concourse.mybir as mybir  # noqa: E402
import concourse.tile as tile  # noqa: E402
from concourse.bass_utils import run_bass_kernel_spmd  # noqa: E402

B, S, D = 2, 2048, 2048
RANK = 512
NH = 16
HG = 4              # heads per core / group size
DHG = D // HG       # 512 cols per core
P = 128
DB = D // P         # 16 d-blocks
RB = RANK // P      # 4 rank-blocks (== own-col blocks)
SC = S // 4         # 512: s-chunk width == t-chunk width
NT = S // SC        # 4 chunks

F32 = mybir.dt.float32
F32R = mybir.dt.float32r
BF = mybir.dt.bfloat16
AF = mybir.ActivationFunctionType
ALU = mybir.AluOpType

EXP_SCALE = 1.0 / float(np.sqrt(P))
INV_KEEP2 = 1.0 / (0.9 * 0.9)
RGROUPS = [[0, 1, 2, 3], [4, 5, 6, 7]]

_CACHE = {}


def _r(ap):
    """[ (o p), f ] DRAM tensor -> [p, o, f] partition-tiled view."""
    return ap.rearrange("(o p) f -> p o f", p=P)


def _build():
    nc = bacc.Bacc("TRN2", target_bir_lowering=False, debug=False,
                   enable_asserts=False, num_devices=8)
    xT = nc.dram_tensor("xT", [D, S], BF, kind="ExternalInput").ap()
    gqT = nc.dram_tensor("gqT", [RANK, S], BF, kind="ExternalInput").ap()
    gkT = nc.dram_tensor("gkT", [RANK, S], BF, kind="ExternalInput").ap()
    gvT = nc.dram_tensor("gvT", [RANK, S], BF, kind="ExternalInput").ap()
    qk_readT = nc.dram_tensor("qk_readT", [D, RANK], BF, kind="ExternalInput").ap()
    v_readT = nc.dram_tensor("v_readT", [D, RANK], BF, kind="ExternalInput").ap()
    qk_w = nc.dram_tensor("qk_w", [RANK, DHG], BF, kind="ExternalInput").ap()
    v_w = nc.dram_tensor("v_w", [RANK, DHG], BF, kind="ExternalInput").ap()
    wo_rows = nc.dram_tensor("wo_rows", [DHG, D], BF, kind="ExternalInput").ap()
    out = nc.dram_tensor("out", [NT, P, D], BF, kind="ExternalOutput").ap()

    with tile.TileContext(nc) as tc:
        _body(tc, xT, gqT, gkT, gvT, qk_readT, v_readT, qk_w, v_w, wo_rows, out)
    nc.compile()
    return nc


def _body(tc, xT, gqT, gkT, gvT, qk_readT, v_readT, qk_w, v_w, wo_rows, out):
    nc = tc.nc
    import contextlib
    ctx = contextlib.ExitStack()
    with ctx:
        pool_main = ctx.enter_context(tc.tile_pool(name="main", bufs=1))
        pool_x = ctx.enter_context(tc.tile_pool(name="x", bufs=1))
        pool_g = ctx.enter_context(tc.tile_pool(name="g", bufs=2))
        pool_tg = ctx.enter_context(tc.tile_pool(name="tg", bufs=1))
        pool_ao = ctx.enter_context(tc.tile_pool(name="ao", bufs=2))
        pool_et = ctx.enter_context(tc.tile_pool(name="et", bufs=6))
        pool_ets = ctx.enter_context(tc.tile_pool(name="ets", bufs=3))
        pool_sm = ctx.enter_context(tc.tile_pool(name="sm", bufs=2))
        pool_rsin = ctx.enter_context(tc.tile_pool(name="rsin", bufs=2))
        pool_dram = ctx.enter_context(tc.tile_pool(name="dramb", bufs=1,
                                                   space="DRAM"))
        psAB = ctx.enter_context(tc.tile_pool(name="psAB", bufs=2, space="PSUM"))
        psSC = ctx.enter_context(tc.tile_pool(name="psSC", bufs=3, space="PSUM"))
        psPV = ctx.enter_context(tc.tile_pool(name="psPV", bufs=2, space="PSUM"))
        psRS = ctx.enter_context(tc.tile_pool(name="psRS", bufs=1, space="PSUM"))

        # ---- long-lived tensors / constants
        QT_sb = pool_main.tile([P, HG, NT, SC], BF)   # Q^T [dh, head, chunk, s]
        KT_sb = pool_main.tile([P, HG, NT, SC], BF)
        V_sb = pool_main.tile([P, DB, DHG], BF)       # V [s-block, own cols]
        wo_sb = pool_main.tile([P, RB, D], BF)        # W_O own rows
        qr = pool_main.tile([P, DB, RANK], BF)
        vr = pool_main.tile([P, DB, RANK], BF)
        qw = pool_main.tile([P, RB, DHG], BF)
        vw = pool_main.tile([P, RB, DHG], BF)
        masks = pool_main.tile([P, HG, SC], BF)
        ones_r = pool_main.tile([P, 1], BF)
        onecol = pool_main.tile([1, P], F32)
        onecol_b = pool_main.tile([1, P], BF)

        nc.sync.dma_start(qr[:, :DB // 2, :], _r(qk_readT)[:, :DB // 2, :])
        nc.sync.dma_start(qr[:, DB // 2:, :], _r(qk_readT)[:, DB // 2:, :])
        nc.vector.memset(masks[:], 1.0)
        for o in range(HG):
            nc.gpsimd.affine_select(
                out=masks[:, o, :], in_=masks[:, o, :],
                compare_op=ALU.is_ge, fill=0.0, base=-P * o,
                pattern=[[1, SC]], channel_multiplier=-1)
        nc.vector.memset(ones_r[:], 1.0)
        nc.vector.memset(onecol[:], 1.0)
        nc.vector.memset(onecol_b[:], 1.0)

        rs_in = pool_dram.tile([NT, SC, D], BF)
        rout = pool_dram.tile([NT, P, D], BF)

        for t in range(NT):
            csl = slice(t * SC, (t + 1) * SC)
            # ---- A(t): gated low-rank t^T for s-chunk t
            xt = pool_x.tile([P, DB, SC], BF, tag="xt")
            nc.sync.dma_start(xt[:, :DB // 2, :], _r(xT)[:, :DB // 2, csl])
            nc.sync.dma_start(xt[:, DB // 2:, :], _r(xT)[:, DB // 2:, csl])
            gq = pool_g.tile([P, RB, SC], BF, tag="gq")
            nc.sync.dma_start(gq[:], _r(gqT)[:, :, csl])
            gk = pool_g.tile([P, RB, SC], BF, tag="gk")
            nc.sync.dma_start(gk[:], _r(gkT)[:, :, csl])
            gv = pool_g.tile([P, RB, SC], BF, tag="gv")
            nc.sync.dma_start(gv[:], _r(gvT)[:, :, csl])
            if t == 0:
                # deferred weight loads: behind chunk-0's x/gates in queue
                # order so the first A matmul isn't starved
                nc.sync.dma_start(vr[:], _r(v_readT))
                nc.sync.dma_start(qw[:], _r(qk_w))
                nc.sync.dma_start(vw[:], _r(v_w))
                nc.sync.dma_start(wo_sb[:], _r(wo_rows))
            tqg = pool_tg.tile([P, RB, SC], BF, tag="tqg")
            tkg = pool_tg.tile([P, RB, SC], BF, tag="tkg")
            tvg = pool_tg.tile([P, RB, SC], BF, tag="tvg")
            for rb in range(RB):
                ps = psAB.tile([P, SC], F32, tag="ab")
                for db in range(DB):
                    nc.tensor.matmul(ps[:], qr[:, db, rb * P:(rb + 1) * P],
                                     xt[:, db, :], start=(db == 0),
                                     stop=(db == DB - 1))
                nc.vector.tensor_tensor(tqg[:, rb, :], ps[:], gq[:, rb, :],
                                        ALU.mult)
                nc.vector.tensor_tensor(tkg[:, rb, :], ps[:], gk[:, rb, :],
                                        ALU.mult)
            for rb in range(RB):
                ps = psAB.tile([P, SC], F32, tag="ab")
                for db in range(DB):
                    nc.tensor.matmul(ps[:], vr[:, db, rb * P:(rb + 1) * P],
                                     xt[:, db, :], start=(db == 0),
                                     stop=(db == DB - 1))
                nc.vector.tensor_tensor(tvg[:, rb, :], ps[:], gv[:, rb, :],
                                        ALU.mult)
            # ---- B(t): Q^T/K^T [own cols, chunk t], V [chunk t, own cols]
            for db in range(RB):
                dsl = slice(db * P, (db + 1) * P)
                psq = psAB.tile([P, SC], F32, tag="ab")
                for rb in range(RB):
                    nc.tensor.matmul(psq[:], qw[:, rb, dsl], tqg[:, rb, :],
                                     start=(rb == 0), stop=(rb == RB - 1))
                nc.scalar.activation(QT_sb[:, db, t, :], psq[:], AF.Copy)
                psk = psAB.tile([P, SC], F32, tag="ab")
                for rb in range(RB):
                    nc.tensor.matmul(psk[:], qw[:, rb, dsl], tkg[:, rb, :],
                                     start=(rb == 0), stop=(rb == RB - 1))
                nc.scalar.activation(KT_sb[:, db, t, :], psk[:], AF.Copy)
            for sj in range(RB):
                sb = t * RB + sj
                ssl2 = slice(sj * P, (sj + 1) * P)
                psv = psAB.tile([P, DHG], F32, tag="ab")
                for rb in range(RB):
                    nc.tensor.matmul(psv[:], tvg[:, rb, ssl2], vw[:, rb, :],
                                     start=(rb == 0), stop=(rb == RB - 1))
                nc.scalar.activation(V_sb[:, sb, :], psv[:], AF.Copy)

            # ---- C(t): attention for queries in chunk t, all own heads
            ao = pool_ao.tile([P, HG, SC], BF, tag="ao")
            npair = 2 * (t + 1)
            nquad = npair // 2

            def head_tail(h, pv, rs, e2args):
                """Finish head h: last quad rowsum, fast fp32 1/Z on DVE,
                f32r broadcast matmul, normalize. 1/0.81 is folded into W_O
                on the host."""
                e2, st, sp = e2args
                nc.tensor.matmul(rs[:], ones_r[:], e2[:], start=st, stop=sp)
                recip = pool_sm.tile([1, SC], F32, tag="recip")
                nc.vector.reciprocal_approx_fast(out=recip[:], in_=rs[:])
                recb = pool_sm.tile([1, SC], BF, tag="recb")
                nc.vector.tensor_copy(recb[:], recip[:])
                rep = psSC.tile([P, SC], F32, tag="sc")
                nc.tensor.matmul(rep[:], onecol_b[:], recb[:],
                                 start=True, stop=True)
                nc.scalar.activation(ao[:, h, :], pv[:], AF.Copy)
                nc.vector.tensor_tensor(ao[:, h, :], ao[:, h, :], rep[:],
                                        ALU.mult)

            prev_tail = None
            for h in range(HG):
                hsl = slice(h * P, (h + 1) * P)
                pv = psPV.tile([P, SC], F32, tag="pv")
                rs = psRS.tile([1, SC], F32, tag="rs")
                pend_rs = []    # one-pair-lagged quad rowsum matmuls
                ets_hold = None

                def sc_pair(q):
                    """Emit scores+exp(+mask) for pair q; return et tiles."""
                    etps = []
                    for k in range(2):
                        jb = 2 * q + k
                        jc, jp = divmod(jb, RB)
                        sc = psSC.tile([P, SC], F32, tag="sc")
                        nc.tensor.matmul(
                            sc[:], KT_sb[:, h, jc, jp * P:(jp + 1) * P],
                            QT_sb[:, h, t, :], start=True, stop=True)
                        etp = pool_et.tile([P, SC], BF, tag="et")
                        nc.scalar.activation(etp[:], sc[:], AF.Exp,
                                             scale=EXP_SCALE)
                        o = jb - 4 * t
                        if o >= 0:
                            nc.vector.tensor_tensor(etp[:], etp[:],
                                                    masks[:, o, :], ALU.mult)
                        etps.append(etp)
                    return etps

                # scores run one pair ahead of PV so the exp latency is
                # hidden behind the previous pair's PV matmuls
                etp_cur = sc_pair(0)
                for q in range(npair):
                    etp_next = sc_pair(q + 1) if q + 1 < npair else None
                    if pend_rs:
                        e2, st, sp = pend_rs.pop()
                        nc.tensor.matmul(rs[:], ones_r[:], e2[:],
                                         start=st, stop=sp)
                    for k in range(2):
                        jb = 2 * q + k
                        nc.tensor.matmul(pv[:], V_sb[:, jb, hsl],
                                         etp_cur[k][:],
                                         start=(q == 0 and k == 0),
                                         stop=(q == npair - 1 and k == 1))
                    if prev_tail is not None:
                        head_tail(*prev_tail)   # overlap prior head's tail
                        prev_tail = None
                    ets = pool_ets.tile([P, SC], BF, tag="ets")
                    nc.vector.tensor_tensor(ets[:], etp_cur[0][:],
                                            etp_cur[1][:], ALU.add)
                    if q % 2 == 0:
                        ets_hold = ets
                    else:
                        qd = q // 2
                        ets2 = pool_ets.tile([P, SC], BF, tag="ets2")
                        nc.vector.tensor_tensor(ets2[:], ets_hold[:], ets[:],
                                                ALU.add)
                        pend_rs.append((ets2, qd == 0, qd == nquad - 1))
                    etp_cur = etp_next
                prev_tail = (h, pv, rs, pend_rs.pop())
            # ---- D(t): partial out rows for this chunk (local ao only)
            for isub in range(4):
                rsin_sb = pool_rsin.tile([P, HG, SC], BF, tag="rsin")
                for oc in range(4):
                    psd = psAB.tile([P, SC], F32, tag="ab")
                    for h in range(HG):
                        if isub == 0 and oc == 0 and h == HG - 1:
                            head_tail(*prev_tail)  # last head tail under D
                            prev_tail = None
                        nc.tensor.matmul(
                            psd[:], ao[:, h, isub * P:(isub + 1) * P],
                            wo_sb[:, h, oc * SC:(oc + 1) * SC],
                            start=(h == 0), stop=(h == HG - 1))
                    nc.vector.tensor_copy(rsin_sb[:, oc, :], psd[:])
                nc.sync.dma_start(
                    _r(rs_in[t])[:, isub, :],
                    rsin_sb[:].rearrange("p o f -> p (o f)"))
            nc.gpsimd.collective_compute(
                "ReduceScatter", ALU.add, ins=[rs_in[t].opt()],
                outs=[rout[t].opt()], replica_groups=RGROUPS)
            nc.sync.dma_start(out[t], rout[t])


def _get_nc():
    if 'nc' not in _CACHE:
        _CACHE['nc'] = _build()
    return _CACHE['nc']


def _bf(a):
    return np.ascontiguousarray(np.asarray(a, np.float32)).astype(
        ml_dtypes.bfloat16)


def kernel(**inputs):
    x = np.asarray(inputs["x"], np.float32)
    g_Q = np.asarray(inputs["g_Q"], np.float32)
    g_K = np.asarray(inputs["g_K"], np.float32)
    g_V = np.asarray(inputs["g_V"], np.float32)
    qk_read = np.asarray(inputs["qk_read"], np.float32)
    qk_write = np.asarray(inputs["qk_write"], np.float32)
    v_read = np.asarray(inputs["v_read"], np.float32)
    v_write = np.asarray(inputs["v_write"], np.float32)
    W_O = np.asarray(inputs["W_O"], np.float32)

    nc = _get_nc()
    qk_readT = _bf(qk_read.T)
    v_readT = _bf(v_read.T)
    xTb = [_bf(x[b].T) for b in range(B)]
    gqTb = [_bf(g_Q[b].T) for b in range(B)]
    gkTb = [_bf(g_K[b].T) for b in range(B)]
    gvTb = [_bf(g_V[b].T) for b in range(B)]
    in_maps = []
    for c in range(8):
        b, g = divmod(c, 4)
        ssl = slice(g * SC, (g + 1) * SC)
        in_maps.append({
            "xT": xTb[b],
            "gqT": gqTb[b],
            "gkT": gkTb[b],
            "gvT": gvTb[b],
            "qk_readT": qk_readT,
            "v_readT": v_readT,
            "qk_w": _bf(qk_write[:, ssl]),
            "v_w": _bf(v_write[:, ssl]),
            "wo_rows": _bf(W_O[ssl, :] * INV_KEEP2),
        })
    res = run_bass_kernel_spmd(nc, in_maps, core_ids=list(range(8)))
    _CACHE['last_results'] = res
    out = np.empty((B, S, D), np.float32)
    for c in range(8):
        b, g = divmod(c, 4)
        o = np.asarray(res.results[c]["out"], dtype=ml_dtypes.bfloat16)
        for t in range(NT):
            r0 = t * SC + g * P
            out[b, r0:r0 + P, :] = o[t].astype(np.float32)
    return out


# revision 32
# speedup vs baseline: 1.0432x; 1.0432x over previous
"""Distributed Bass kernel for nn_AttentionCircuit (B=2,S=2048,D=2048,RANK=512,H=16).

Sharding: 8 cores = 2 batches x 4 group-positions. Core (b, g) computes
attention for head-group g (4 heads / 512 D-cols) of batch b over all S,
then a partial W_O product over its 512 AO columns; a ReduceScatter(add)
per 512-row chunk sums the partials and scatters rows across the group.
All matmul operands bf16 (1 cyc/row on PE, half the DMA bytes of fp32).

Per-core dataflow, streamed per 512-wide s-chunk c (no AllGather: the gated
low-rank t is recomputed locally from x, which keeps the PE continuously
busy instead of idling ~100us on a collective):
  A(c): t^T = read @ x_c^T, gate with g_c^T -> tqg/tkg/tvg [rank, 512]
  B(c): Q^T/K^T[own cols, chunk c], V[chunk c, own cols]
  C(t=c): per head: scores^T = K^T.T Q^T -> exp -> causal mask (block skip +
     static masks on diagonal) -> rowsum via ones-matmul of DVE pair-sums ->
     PV matmul -> normalize with outer(1/0.81, 1/rowsum) bcast matmul
  D(c): partial out rows = ao^T.T @ W_O[own rows, :]
  RS(c): ReduceScatter(add) over the 4-core group -> 128 output rows.
Host reassembles: core (b, g) holds rows t*512+g*128..+128 of batch b.
"""
import sys
import numpy as np
import ml_dtypes

sys.path.insert(0, '/opt/trn_rl_repo')

import concourse.bass as bass  # noqa: E402
from concourse import bacc  # noqa: E402
import concourse.mybir as mybir  # noqa: E402
import concourse.tile as tile  # noqa: E402
from concourse.bass_utils import run_bass_kernel_spmd  # noqa: E402

B, S, D = 2, 2048, 2048
RANK = 512
NH = 16
HG = 4              # heads per core / group size
DHG = D // HG       # 512 cols per core
P = 128
DB = D // P         # 16 d-blocks
RB = RANK // P      # 4 rank-blocks (== own-col blocks)
SC = S // 4         # 512: s-chunk width == t-chunk width
NT = S // SC        # 4 chunks

F32 = mybir.dt.float32
F32R = mybir.dt.float32r
BF = mybir.dt.bfloat16
AF = mybir.ActivationFunctionType
ALU = mybir.AluOpType

EXP_SCALE = 1.0 / float(np.sqrt(P))
INV_KEEP2 = 1.0 / (0.9 * 0.9)
RGROUPS = [[0, 1, 2, 3], [4, 5, 6, 7]]

_CACHE = {}


def _r(ap):
    """[ (o p), f ] DRAM tensor -> [p, o, f] partition-tiled view."""
    return ap.rearrange("(o p) f -> p o f", p=P)


def _build():
    nc = bacc.Bacc("TRN2", target_bir_lowering=False, debug=False,
                   enable_asserts=False, num_devices=8)
    xT = nc.dram_tensor("xT", [D, S], BF, kind="ExternalInput").ap()
    gqT = nc.dram_tensor("gqT", [RANK, S], BF, kind="ExternalInput").ap()
    gkT = nc.dram_tensor("gkT", [RANK, S], BF, kind="ExternalInput").ap()
    gvT = nc.dram_tensor("gvT", [RANK, S], BF, kind="ExternalInput").ap()
    qk_readT = nc.dram_tensor("qk_readT", [D, RANK], BF, kind="ExternalInput").ap()
    v_readT = nc.dram_tensor("v_readT", [D, RANK], BF, kind="ExternalInput").ap()
    qk_w = nc.dram_tensor("qk_w", [RANK, DHG], BF, kind="ExternalInput").ap()
    v_w = nc.dram_tensor("v_w", [RANK, DHG], BF, kind="ExternalInput").ap()
    wo_rows = nc.dram_tensor("wo_rows", [DHG, D], BF, kind="ExternalInput").ap()
    out = nc.dram_tensor("out", [NT, P, D], BF, kind="ExternalOutput").ap()

    with tile.TileContext(nc) as tc:
        _body(tc, xT, gqT, gkT, gvT, qk_readT, v_readT, qk_w, v_w, wo_rows, out)
    nc.compile()
    return nc


def _body(tc, xT, gqT, gkT, gvT, qk_readT, v_readT, qk_w, v_w, wo_rows, out):
    nc = tc.nc
    import contextlib
    ctx = contextlib.ExitStack()
    with ctx:
        pool_main = ctx.enter_context(tc.tile_pool(name="main", bufs=1))
        pool_x = ctx.enter_context(tc.tile_pool(name="x", bufs=1))
        pool_g = ctx.enter_context(tc.tile_pool(name="g", bufs=2))
        pool_tg = ctx.enter_context(tc.tile_pool(name="tg", bufs=1))
        pool_ao = ctx.enter_context(tc.tile_pool(name="ao", bufs=2))
        pool_et = ctx.enter_context(tc.tile_pool(name="et", bufs=6))
        pool_ets = ctx.enter_context(tc.tile_pool(name="ets", bufs=3))
        pool_sm = ctx.enter_context(tc.tile_pool(name="sm", bufs=2))
        pool_rsin = ctx.enter_context(tc.tile_pool(name="rsin", bufs=2))
        pool_dram = ctx.enter_context(tc.tile_pool(name="dramb", bufs=1,
                                                   space="DRAM"))
        psAB = ctx.enter_context(tc.tile_pool(name="psAB", bufs=2, space="PSUM"))
        psSC = ctx.enter_context(tc.tile_pool(name="psSC", bufs=3, space="PSUM"))
        psPV = ctx.enter_context(tc.tile_pool(name="psPV", bufs=2, space="PSUM"))
        psRS = ctx.enter_context(tc.tile_pool(name="psRS", bufs=1, space="PSUM"))

        # ---- long-lived tensors / constants
        QT_sb = pool_main.tile([P, HG, NT, SC], BF)   # Q^T [dh, head, chunk, s]
        KT_sb = pool_main.tile([P, HG, NT, SC], BF)
        V_sb = pool_main.tile([P, DB, DHG], BF)       # V [s-block, own cols]
        wo_sb = pool_main.tile([P, RB, D], BF)        # W_O own rows
        qr = pool_main.tile([P, DB, RANK], BF)
        vr = pool_main.tile([P, DB, RANK], BF)
        qw = pool_main.tile([P, RB, DHG], BF)
        vw = pool_main.tile([P, RB, DHG], BF)
        masks = pool_main.tile([P, HG, SC], BF)
        ones_r = pool_main.tile([P, 1], BF)
        onecol = pool_main.tile([1, P], F32)

        nc.sync.dma_start(qr[:, :DB // 2, :], _r(qk_readT)[:, :DB // 2, :])
        nc.sync.dma_start(qr[:, DB // 2:, :], _r(qk_readT)[:, DB // 2:, :])
        nc.vector.memset(masks[:], 1.0)
        for o in range(HG):
            nc.gpsimd.affine_select(
                out=masks[:, o, :], in_=masks[:, o, :],
                compare_op=ALU.is_ge, fill=0.0, base=-P * o,
                pattern=[[1, SC]], channel_multiplier=-1)
        nc.vector.memset(ones_r[:], 1.0)
        nc.vector.memset(onecol[:], 1.0)

        rs_in = pool_dram.tile([NT, SC, D], BF)
        rout = pool_dram.tile([NT, P, D], BF)

        for t in range(NT):
            csl = slice(t * SC, (t + 1) * SC)
            # ---- A(t): gated low-rank t^T for s-chunk t
            xt = pool_x.tile([P, DB, SC], BF, tag="xt")
            nc.sync.dma_start(xt[:, :DB // 2, :], _r(xT)[:, :DB // 2, csl])
            nc.sync.dma_start(xt[:, DB // 2:, :], _r(xT)[:, DB // 2:, csl])
            gq = pool_g.tile([P, RB, SC], BF, tag="gq")
            nc.sync.dma_start(gq[:], _r(gqT)[:, :, csl])
            gk = pool_g.tile([P, RB, SC], BF, tag="gk")
            nc.sync.dma_start(gk[:], _r(gkT)[:, :, csl])
            gv = pool_g.tile([P, RB, SC], BF, tag="gv")
            nc.sync.dma_start(gv[:], _r(gvT)[:, :, csl])
            if t == 0:
                # deferred weight loads: behind chunk-0's x/gates in queue
                # order so the first A matmul isn't starved
                nc.sync.dma_start(vr[:], _r(v_readT))
                nc.sync.dma_start(qw[:], _r(qk_w))
                nc.sync.dma_start(vw[:], _r(v_w))
                nc.sync.dma_start(wo_sb[:], _r(wo_rows))
            tqg = pool_tg.tile([P, RB, SC], BF, tag="tqg")
            tkg = pool_tg.tile([P, RB, SC], BF, tag="tkg")
            tvg = pool_tg.tile([P, RB, SC], BF, tag="tvg")
            for rb in range(RB):
                ps = psAB.tile([P, SC], F32, tag="ab")
                for db in range(DB):
                    nc.tensor.matmul(ps[:], qr[:, db, rb * P:(rb + 1) * P],
                                     xt[:, db, :], start=(db == 0),
                                     stop=(db == DB - 1))
                nc.vector.tensor_tensor(tqg[:, rb, :], ps[:], gq[:, rb, :],
                                        ALU.mult)
                nc.vector.tensor_tensor(tkg[:, rb, :], ps[:], gk[:, rb, :],
                                        ALU.mult)
            for rb in range(RB):
                ps = psAB.tile([P, SC], F32, tag="ab")
                for db in range(DB):
                    nc.tensor.matmul(ps[:], vr[:, db, rb * P:(rb + 1) * P],
                                     xt[:, db, :], start=(db == 0),
                                     stop=(db == DB - 1))
                nc.vector.tensor_tensor(tvg[:, rb, :], ps[:], gv[:, rb, :],
                                        ALU.mult)
            # ---- B(t): Q^T/K^T [own cols, chunk t], V [chunk t, own cols]
            for db in range(RB):
                dsl = slice(db * P, (db + 1) * P)
                psq = psAB.tile([P, SC], F32, tag="ab")
                for rb in range(RB):
                    nc.tensor.matmul(psq[:], qw[:, rb, dsl], tqg[:, rb, :],
                                     start=(rb == 0), stop=(rb == RB - 1))
                nc.scalar.activation(QT_sb[:, db, t, :], psq[:], AF.Copy)
                psk = psAB.tile([P, SC], F32, tag="ab")
                for rb in range(RB):
                    nc.tensor.matmul(psk[:], qw[:, rb, dsl], tkg[:, rb, :],
                                     start=(rb == 0), stop=(rb == RB - 1))
                nc.scalar.activation(KT_sb[:, db, t, :], psk[:], AF.Copy)
            for sj in range(RB):
                sb = t * RB + sj
                ssl2 = slice(sj * P, (sj + 1) * P)
                psv = psAB.tile([P, DHG], F32, tag="ab")
                for rb in range(RB):
                    nc.tensor.matmul(psv[:], tvg[:, rb, ssl2], vw[:, rb, :],
                                     start=(rb == 0), stop=(rb == RB - 1))
                nc.scalar.activation(V_sb[:, sb, :], psv[:], AF.Copy)

            # ---- C(t): attention for queries in chunk t, all own heads
            ao = pool_ao.tile([P, HG, SC], BF, tag="ao")
            npair = 2 * (t + 1)
            nquad = npair // 2

            def head_tail(h, pv, rs, e2args):
                """Finish head h: last quad rowsum, fast fp32 1/Z on DVE,
                f32r broadcast matmul, normalize. 1/0.81 is folded into W_O
                on the host."""
                e2, st, sp = e2args
                nc.tensor.matmul(rs[:], ones_r[:], e2[:], start=st, stop=sp)
                recip = pool_sm.tile([1, SC], F32, tag="recip")
                nc.vector.reciprocal_approx_fast(out=recip[:], in_=rs[:])
                rep = psSC.tile([P, SC], F32, tag="sc")
                nc.tensor.matmul(rep[:], onecol[:], recip[:],
                                 start=True, stop=True)
                nc.scalar.activation(ao[:, h, :], pv[:], AF.Copy)
                nc.vector.tensor_tensor(ao[:, h, :], ao[:, h, :], rep[:],
                                        ALU.mult)

            prev_tail = None
            for h in range(HG):
                hsl = slice(h * P, (h + 1) * P)
                pv = psPV.tile([P, SC], F32, tag="pv")
                rs = psRS.tile([1, SC], F32, tag="rs")
                pend_rs = []    # one-pair-lagged quad rowsum matmuls
                ets_hold = None

                def sc_pair(q):
                    """Emit scores+exp(+mask) for pair q; return et tiles."""
                    etps = []
                    for k in range(2):
                        jb = 2 * q + k
                        jc, jp = divmod(jb, RB)
                        sc = psSC.tile([P, SC], F32, tag="sc")
                        nc.tensor.matmul(
                            sc[:], KT_sb[:, h, jc, jp * P:(jp + 1) * P],
                            QT_sb[:, h, t, :], start=True, stop=True)
                        etp = pool_et.tile([P, SC], BF, tag="et")
                        nc.scalar.activation(etp[:], sc[:], AF.Exp,
                                             scale=EXP_SCALE)
                        o = jb - 4 * t
                        if o >= 0:
                            nc.vector.tensor_tensor(etp[:], etp[:],
                                                    masks[:, o, :], ALU.mult)
                        etps.append(etp)
                    return etps

                # scores run one pair ahead of PV so the exp latency is
                # hidden behind the previous pair's PV matmuls
                etp_cur = sc_pair(0)
                for q in range(npair):
                    etp_next = sc_pair(q + 1) if q + 1 < npair else None
                    if pend_rs:
                        e2, st, sp = pend_rs.pop()
                        nc.tensor.matmul(rs[:], ones_r[:], e2[:],
                                         start=st, stop=sp)
                    for k in range(2):
                        jb = 2 * q + k
                        nc.tensor.matmul(pv[:], V_sb[:, jb, hsl],
                                         etp_cur[k][:],
                                         start=(q == 0 and k == 0),
                                         stop=(q == npair - 1 and k == 1))
                    if prev_tail is not None:
                        head_tail(*prev_tail)   # overlap prior head's tail
                        prev_tail = None
                    ets = pool_ets.tile([P, SC], BF, tag="ets")
                    nc.vector.tensor_tensor(ets[:], etp_cur[0][:],
                                            etp_cur[1][:], ALU.add)
                    if q % 2 == 0:
                        ets_hold = ets
                    else:
                        qd = q // 2
                        ets2 = pool_ets.tile([P, SC], BF, tag="ets2")
                        nc.vector.tensor_tensor(ets2[:], ets_hold[:], ets[:],
                                                ALU.add)
                        pend_rs.append((ets2, qd == 0, qd == nquad - 1))
                    etp_cur = etp_next
                prev_tail = (h, pv, rs, pend_rs.pop())
            # ---- D(t): partial out rows for this chunk (local ao only)
            for isub in range(4):
                rsin_sb = pool_rsin.tile([P, HG, SC], BF, tag="rsin")
                for oc in range(4):
                    psd = psAB.tile([P, SC], F32, tag="ab")
                    for h in range(HG):
                        if isub == 0 and oc == 0 and h == HG - 1:
                            head_tail(*prev_tail)  # last head tail under D
                            prev_tail = None
                        nc.tensor.matmul(
                            psd[:], ao[:, h, isub * P:(isub + 1) * P],
                            wo_sb[:, h, oc * SC:(oc + 1) * SC],
                            start=(h == 0), stop=(h == HG - 1))
                    nc.vector.tensor_copy(rsin_sb[:, oc, :], psd[:])
                nc.sync.dma_start(
                    _r(rs_in[t])[:, isub, :],
                    rsin_sb[:].rearrange("p o f -> p (o f)"))
            nc.gpsimd.collective_compute(
                "ReduceScatter", ALU.add, ins=[rs_in[t].opt()],
                outs=[rout[t].opt()], replica_groups=RGROUPS)
            nc.sync.dma_start(out[t], rout[t])


def _get_nc():
    if 'nc' not in _CACHE:
        _CACHE['nc'] = _build()
    return _CACHE['nc']


def _bf(a):
    return np.ascontiguousarray(np.asarray(a, np.float32)).astype(
        ml_dtypes.bfloat16)


def kernel(**inputs):
    x = np.asarray(inputs["x"], np.float32)
    g_Q = np.asarray(inputs["g_Q"], np.float32)
    g_K = np.asarray(inputs["g_K"], np.float32)
    g_V = np.asarray(inputs["g_V"], np.float32)
    qk_read = np.asarray(inputs["qk_read"], np.float32)
    qk_write = np.asarray(inputs["qk_write"], np.float32)
    v_read = np.asarray(inputs["v_read"], np.float32)
    v_write = np.asarray(inputs["v_write"], np.float32)
    W_O = np.asarray(inputs["W_O"], np.float32)

    nc = _get_nc()
    qk_readT = _bf(qk_read.T)
    v_readT = _bf(v_read.T)
    xTb = [_bf(x[b].T) for b in range(B)]
    gqTb = [_bf(g_Q[b].T) for b in range(B)]
    gkTb = [_bf(g_K[b].T) for b in range(B)]
    gvTb = [_bf(g_V[b].T) for b in range(B)]
    in_maps = []
    for c in range(8):
        b, g = divmod(c, 4)
        ssl = slice(g * SC, (g + 1) * SC)
        in_maps.append({
            "xT": xTb[b],
            "gqT": gqTb[b],
            "gkT": gkTb[b],
            "gvT": gvTb[b],
            "qk_readT": qk_readT,
            "v_readT": v_readT,
            "qk_w": _bf(qk_write[:, ssl]),
            "v_w": _bf(v_write[:, ssl]),
            "wo_rows": _bf(W_O[ssl, :] * INV_KEEP2),
        })
    res = run_bass_kernel_spmd(nc, in_maps, core_ids=list(range(8)))
    _CACHE['last_results'] = res
    out = np.empty((B, S, D), np.float32)
    for c in range(8):
        b, g = divmod(c, 4)
        o = np.asarray(res.results[c]["out"], dtype=ml_dtypes.bfloat16)
        for t in range(NT):
            r0 = t * SC + g * P
            out[b, r0:r0 + P, :] = o[t].astype(np.float32)
    return out
